# revision 4
# baseline (speedup 1.0000x reference)
"""Trainium2 Bass kernel for LocalXLAttention (chunk-summed variant).

Math: the reference einsum sums over the chunk index z, so every query
attends to the same three [w, dh] K/V matrices built from chunk sums:
  K_prev = S_k - k_chunk[C-1], K_cur = S_k, K_next = S_k - k_chunk[0]
(and identically for V), where S_k = sum_c k_chunk[c].  The computation
collapses to, per sequence position l and head h:
  attn[l,h,:]  = qp[l,h,:] @ KbigT          (KbigT: [dh, 3w])
  probs        = softmax(attn, axis=-1)
  ctx[l,h,:]   = probs[l,h,:] @ Vbig        (Vbig:  [3w, dh])
  out          = ctx.reshape(L, dm) @ Wc

Sharding: L=4096 is split 512 rows per core across 8 NeuronCores
(data-parallel over the sequence; no collectives).  Each core redundantly
computes the tiny chunk-summed K/V from the full kv input.

All matmul operands are bf16 (fp32 PSUM accumulation): this enables the
PE fast-weight-load path (fp32 LDWEIGHTS was the previous bottleneck),
halves HBM traffic, and halves DVE element cost.  The attention pipeline
runs fully transposed ([j, l] / [he, l] layouts); probs normalization is
deferred to the context (an extra all-ones column of Vbig accumulates the
softmax denominator for free).

Schedule highlights:
 - PE warm-up matmuls during the initial DMA wait (HAM clock gate).
 - exp table preloaded during startup; exp runs as [128, 4096]
   activations from an SBUF staging buffer (DVE evacuates qk PSUM),
   amortizing the per-instruction ACT overhead.
 - DMA priority: Wq/q first (QP_T starts ~9us in), kv stream next,
   Wc during attention, outputs streamed per 128-row chunk.
"""

import sys
for _p in ('/opt/pypackages', '/opt/trn_rl_repo'):
    if _p not in sys.path:
        sys.path.insert(0, _p)

import numpy as np
import ml_dtypes

import concourse.bass as bass
import concourse.bacc as bacc
import concourse.tile as tile
from concourse import mybir
from concourse.bass_utils import run_bass_kernel_spmd
from concourse.masks import make_identity

F32 = mybir.dt.float32
BF16 = mybir.dt.bfloat16
AF = mybir.ActivationFunctionType

N_CORES = 8
L = 4096          # full sequence
LS = L // N_CORES # 512 rows per core
DM = 1024
NH = 16
DH = 64
W = 512           # chunk width
C = L // W        # 8 chunks
J3 = 3 * W        # 1536 softmax width
NJ = J3 // 128    # 12 j-chunks
DMT = DM // 128   # 8 dm-chunks
NPAIR = NH // 2   # 8 head pairs
JG = 4            # j-chunks per exp group
NG = NJ // JG     # 3 exp groups per pair


def build_nc():
    nc = bacc.Bacc(None, target_bir_lowering=False)

    qT = nc.dram_tensor("qT", [DM, LS], BF16, kind="ExternalInput")
    kvT = nc.dram_tensor("kvT", [DM, L], BF16, kind="ExternalInput")
    Wq = nc.dram_tensor("Wq", [DM, DM], BF16, kind="ExternalInput")
    Wkv = nc.dram_tensor("Wkv", [DM, 2 * DH], BF16, kind="ExternalInput")
    Wc = nc.dram_tensor("Wc", [DM, DM], BF16, kind="ExternalInput")
    out = nc.dram_tensor("out", [LS, DM], F32, kind="ExternalOutput")

    dma_engines = None  # set inside context

    with tile.TileContext(nc) as tc:
        with tc.tile_pool(name="weights", bufs=8) as wpool, \
             tc.tile_pool(name="small", bufs=1) as spool, \
             tc.tile_pool(name="qp", bufs=8) as qpool, \
             tc.tile_pool(name="stream", bufs=2) as stpool, \
             tc.tile_pool(name="kvsum", bufs=3) as kvspool, \
             tc.tile_pool(name="qkbuf", bufs=2) as qkpool, \
             tc.tile_pool(name="probs", bufs=2) as ppool, \
             tc.tile_pool(name="misc", bufs=2) as mpool, \
             tc.tile_pool(name="dram", bufs=1, space="DRAM") as dpool, \
             tc.tile_pool(name="psacc", bufs=4, space="PSUM") as psacc, \
             tc.tile_pool(name="psmm", bufs=2, space="PSUM") as psmm:

            ENGS = (nc.sync, nc.scalar, nc.gpsimd)

            # ---------- priority loads: Wq, Wkv, qT (QP dependencies) ----
            wq_sb = []
            for d in range(DMT):
                t = wpool.tile([128, DM], BF16, tag="wq", name=f"wq{d}")
                ENGS[d % 3].dma_start(out=t, in_=Wq[128 * d:128 * (d + 1), :])
                wq_sb.append(t)
            wkv_sb = []
            for d in range(DMT):
                t = wpool.tile([128, 2 * DH], BF16, tag="wkv", name=f"wkv{d}")
                ENGS[d % 3].dma_start(out=t, in_=Wkv[128 * d:128 * (d + 1), :])
                wkv_sb.append(t)
            qt_sb = []
            for d in range(DMT):
                t = qpool.tile([128, LS], BF16, tag="qt", name=f"qt{d}")
                ENGS[(d + 1) % 3].dma_start(out=t, in_=qT[128 * d:128 * (d + 1), :])
                qt_sb.append(t)

            ident = spool.tile([128, 128], BF16, tag="ident")
            make_identity(nc, ident)

            # ---------- PE warm-up during the DMA wait (HAM clock gate) --
            # ~120 back-to-back tiny matmuls keep the PE activity monitor
            # in the K=8/8 state so QP_T starts at 2.4 GHz.  Results are
            # never read; the psacc slot is recycled by the kv phase.
            warm_ps = psacc.tile([128, W], F32, tag="acc", name="warm")
            for i in range(120):
                nc.tensor.matmul(warm_ps[:, 0:128], ident, ident,
                                 start=True, stop=True)

            # preload the exp activation table during startup
            exp_warm = spool.tile([1, 8], F32, tag="expwarm")
            nc.scalar.activation(exp_warm, ident[0:1, 0:8], AF.Exp, scale=1.0)

            # ---------- QP_T = Wq.T @ q.T  (1/sqrt(dh) folded into exp) --
            # qpt_sb[t] [128, LS]: partitions 0:64 head 2t dims,
            # 64:128 head 2t+1 dims; bf16.
            qpt_sb = []
            for tp in range(4):
                ps = psmm.tile([128, 1024], F32, tag="mm")
                for half in range(2):
                    hd = 2 * tp + half
                    for d in range(DMT):
                        nc.tensor.matmul(
                            ps[:, 512 * half:512 * (half + 1)],
                            wq_sb[d][:, 128 * hd:128 * (hd + 1)],
                            qt_sb[d],
                            start=(d == 0), stop=(d == DMT - 1))
                for half in range(2):
                    sb = qpool.tile([128, LS], BF16, tag="qpt",
                                    name=f"qpt{2 * tp + half}")
                    nc.vector.tensor_copy(sb, ps[:, 512 * half:512 * (half + 1)])
                    qpt_sb.append(sb)

            # ---------- kv stream: chunk-sum tree + projections ----------
            # Projections of chunk0 / chunk7 are taken straight off the
            # stream tile before the in-place tree mutates it.
            # PSUM accumulators pack K rows 0:64, V rows 64:128.
            ps0 = psacc.tile([128, W], F32, tag="acc", name="ps0")
            ps7 = psacc.tile([128, W], F32, tag="acc", name="ps7")
            pss = psacc.tile([128, W], F32, tag="acc", name="pss")
            for d in range(DMT):
                st = stpool.tile([128, L], BF16, tag="kvstream")
                ENGS[(2 * d) % 3].dma_start(
                    out=st[:, 0:L // 2],
                    in_=kvT[128 * d:128 * (d + 1), 0:L // 2])
                ENGS[(2 * d + 1) % 3].dma_start(
                    out=st[:, L // 2:L],
                    in_=kvT[128 * d:128 * (d + 1), L // 2:L])
                nc.tensor.matmul(ps0[0:DH, :], wkv_sb[d][:, 0:DH],
                                 st[:, 0:W], start=(d == 0), stop=(d == DMT - 1))
                nc.tensor.matmul(ps0[DH:128, :], wkv_sb[d][:, DH:2 * DH],
                                 st[:, 0:W], start=(d == 0), stop=(d == DMT - 1))
                nc.tensor.matmul(ps7[0:DH, :], wkv_sb[d][:, 0:DH],
                                 st[:, L - W:L], start=(d == 0), stop=(d == DMT - 1))
                nc.tensor.matmul(ps7[DH:128, :], wkv_sb[d][:, DH:2 * DH],
                                 st[:, L - W:L], start=(d == 0), stop=(d == DMT - 1))
                nc.vector.tensor_add(st[:, 0:2048], st[:, 0:2048], st[:, 2048:4096])
                nc.vector.tensor_add(st[:, 0:1024], st[:, 0:1024], st[:, 1024:2048])
                ks = kvspool.tile([128, W], BF16, tag="kvsum")
                nc.vector.tensor_add(ks, st[:, 0:512], st[:, 512:1024])
                nc.tensor.matmul(pss[0:DH, :], wkv_sb[d][:, 0:DH],
                                 ks, start=(d == 0), stop=(d == DMT - 1))
                nc.tensor.matmul(pss[DH:128, :], wkv_sb[d][:, DH:2 * DH],
                                 ks, start=(d == 0), stop=(d == DMT - 1))

            # ---------- Wc loads (queued behind kv on the same queues) ---
            wc_sb = []
            for d in range(DMT):
                t = wpool.tile([128, DM], BF16, tag="wc", name=f"wc{d}")
                ENGS[d % 3].dma_start(out=t, in_=Wc[128 * d:128 * (d + 1), :])
                wc_sb.append(t)

            # ---------- evacuate K/V variants to SBUF (bf16) -------------
            kv0_sb = spool.tile([128, W], BF16, tag="kv0")  # K rows 0:64, V 64:128
            kv7_sb = spool.tile([128, W], BF16, tag="kv7")
            kvs_sb = spool.tile([128, W], BF16, tag="kvs")
            nc.vector.tensor_copy(kv0_sb, ps0)
            nc.vector.tensor_copy(kv7_sb, ps7)
            nc.vector.tensor_copy(kvs_sb, pss)

            # ---------- KbigT [128, 1536] = [prev | cur | next] ----------
            # duplicated into partitions 64:128 for the row-packed QK pair.
            kbig = spool.tile([128, J3], BF16, tag="kbig")
            nc.vector.tensor_sub(kbig[0:DH, 0:W], kvs_sb[0:DH, :], kv7_sb[0:DH, :])
            nc.vector.tensor_copy(kbig[0:DH, W:2 * W], kvs_sb[0:DH, :])
            nc.vector.tensor_sub(kbig[0:DH, 2 * W:3 * W], kvs_sb[0:DH, :],
                                 kv0_sb[0:DH, :])
            nc.vector.tensor_copy(kbig[DH:2 * DH, :], kbig[0:DH, :])

            # ---------- Vbig [128, 12, 65(+pad)] -------------------------
            # j-chunk rows p: key index 128j + p; col 64 = ones (softmax
            # denominator accumulator).
            vbig = spool.tile([128, NJ, 68], BF16, tag="vbig")
            ones_sb = spool.tile([128, 1], BF16, tag="ones")
            nc.vector.memset(ones_sb, 1.0)
            for j in range(NJ):
                nc.vector.tensor_copy(vbig[:, j, DH:DH + 1], ones_sb)
            for yt in range(4):
                tps = psacc.tile([128, DH], BF16, tag="acc")
                tp0 = psacc.tile([128, DH], BF16, tag="acc")
                tp7 = psacc.tile([128, DH], BF16, tag="acc")
                sl = slice(128 * yt, 128 * (yt + 1))
                # V rows live at base partition 64; the identity operand must
                # share that base, and ident[64:128, 64:128] is an identity.
                idq = ident[DH:128, DH:128]
                nc.tensor.transpose(tps, kvs_sb[DH:128, sl], idq)
                nc.tensor.transpose(tp0, kv0_sb[DH:128, sl], idq)
                nc.tensor.transpose(tp7, kv7_sb[DH:128, sl], idq)
                # DVE may read only one PSUM operand: evacuate cur first,
                # then subtract the other transposes against the SBUF copy.
                nc.vector.tensor_copy(vbig[:, 4 + yt, 0:DH], tps)
                nc.vector.tensor_sub(vbig[:, 0 + yt, 0:DH], vbig[:, 4 + yt, 0:DH], tp7)
                nc.vector.tensor_sub(vbig[:, 8 + yt, 0:DH], vbig[:, 4 + yt, 0:DH], tp0)

            # ---------- attention: QK -> (DVE evac) -> exp -> PV ---------
            dscratch = dpool.tile([NH, W], F32, name="dscratch")
            rsc = dpool.tile([NH, W], BF16, name="rsc")
            ctxu_sb = []  # per pair [128, 512]: rows 0:64 head 2t, 64:128 head 2t+1
            for t in range(NPAIR):
                ctxu_sb.append(qpool.tile([128, W], BF16, tag="qt", name=f"ctxu{t}"))

            def normalize_batch(b):
                # heads 8b:8b+8 -> reciprocal -> broadcast -> scale ctxu
                dn = mpool.tile([8, W], F32, tag="dn", name=f"dn{b}", bufs=1)
                nc.gpsimd.dma_start(out=dn, in_=dscratch[8 * b:8 * b + 8, :])
                rc = mpool.tile([8, W], F32, tag="rc", name=f"rc{b}", bufs=1)
                nc.vector.reciprocal_approx_fast(rc, dn)
                rc16 = mpool.tile([8, W], BF16, tag="rc16", name=f"rc16{b}", bufs=1)
                nc.vector.tensor_copy(rc16, rc)
                nc.gpsimd.dma_start(out=rsc[8 * b:8 * b + 8, :], in_=rc16)
                for t in range(4 * b, 4 * b + 4):
                    bc = mpool.tile([128, W], BF16, tag="bcast", name=f"bc{t}")
                    src = bass.AP(tensor=rsc.tensor,
                                  offset=rsc.offset + 2 * t * W,
                                  ap=[[W, 2], [0, DH], [1, W]])
                    nc.gpsimd.dma_start(out=bc, in_=src)
                    nc.vector.tensor_mul(ctxu_sb[t], ctxu_sb[t], bc)

            for t in range(NPAIR):  # head pairs (2t, 2t+1)
                qpt = qpt_sb[t]
                ctxA = psacc.tile([128, W], F32, tag="acc", name=f"ctxA{t}")
                ctxB = psacc.tile([128, W], F32, tag="acc", name=f"ctxB{t}")
                for g in range(NG):
                    qkbuf = qkpool.tile([128, JG * 1024], F32, tag="qkbuf")
                    for jj in range(JG):
                        j = JG * g + jj
                        qk = psmm.tile([128, 1024], F32, tag="mm", name=f"qk{t}_{j}")
                        # row-packed pair: even head on PE rows 0:64, odd
                        # head on rows 64:128 -> back-to-back streams with
                        # resident stationaries.
                        nc.tensor.matmul(qk[:, 0:W],
                                         kbig[0:DH, 128 * j:128 * (j + 1)],
                                         qpt[0:DH, :], start=True, stop=True)
                        nc.tensor.matmul(qk[:, W:2 * W],
                                         kbig[DH:2 * DH, 128 * j:128 * (j + 1)],
                                         qpt[DH:128, :], start=True, stop=True)
                        nc.vector.tensor_copy(
                            qkbuf[:, 1024 * jj:1024 * (jj + 1)], qk)
                    pr = ppool.tile([128, JG * 1024], BF16, tag="probs",
                                    name=f"pr{t}_{g}")
                    nc.scalar.activation(pr, qkbuf, AF.Exp, scale=0.125)
                    for jj in range(JG):
                        j = JG * g + jj
                        nc.tensor.matmul(ctxA[0:DH + 1, :], vbig[:, j, 0:DH + 1],
                                         pr[:, 1024 * jj:1024 * jj + W],
                                         start=(j == 0), stop=(j == NJ - 1))
                        nc.tensor.matmul(ctxB[0:DH + 1, :], vbig[:, j, 0:DH + 1],
                                         pr[:, 1024 * jj + W:1024 * (jj + 1)],
                                         start=(j == 0), stop=(j == NJ - 1))
                # evacuate ctx (cast bf16) + denominator rows
                nc.vector.tensor_copy(ctxu_sb[t][0:DH, :], ctxA[0:DH, :])
                nc.vector.tensor_copy(ctxu_sb[t][DH:128, :], ctxB[0:DH, :])
                dtmp = mpool.tile([1, 2 * W], F32, tag="dtmp", name=f"dtmp{t}",
                                  bufs=2)
                nc.vector.tensor_copy(dtmp[:, 0:W], ctxA[DH:DH + 1, :])
                nc.vector.tensor_copy(dtmp[:, W:2 * W], ctxB[DH:DH + 1, :])
                nc.sync.dma_start(out=dscratch[2 * t:2 * t + 1, :],
                                  in_=dtmp[:, 0:W])
                nc.sync.dma_start(out=dscratch[2 * t + 1:2 * t + 2, :],
                                  in_=dtmp[:, W:2 * W])
                if t == 3:
                    normalize_batch(0)
            normalize_batch(1)

            # ---------- out = ctx @ Wc ----------
            for lt in range(LS // 128):
                ps = psmm.tile([128, 1024], F32, tag="mm")
                for half in range(2):
                    for he in range(DMT):
                        nc.tensor.matmul(
                            ps[:, 512 * half:512 * (half + 1)],
                            ctxu_sb[he][:, 128 * lt:128 * (lt + 1)],
                            wc_sb[he][:, 512 * half:512 * (half + 1)],
                            start=(he == 0), stop=(he == DMT - 1))
                ob = mpool.tile([128, DM], F32, tag="outsb", bufs=2)
                nc.vector.tensor_copy(ob, ps)
                nc.sync.dma_start(out=out[128 * lt:128 * (lt + 1), :], in_=ob)

    nc.compile()
    return nc


_NC = None


def _get_nc():
    global _NC
    if _NC is None:
        _NC = build_nc()
    return _NC


def prep_in_maps(q, kv, Wq, Wkv, Wc):
    """Host-side input prep: transpose, cast to bf16, shard queries."""
    bf16 = ml_dtypes.bfloat16
    qT_full = np.ascontiguousarray(np.asarray(q, dtype=np.float32)[0].T
                                   ).astype(bf16)
    kvT = np.ascontiguousarray(np.asarray(kv, dtype=np.float32)[0].T
                               ).astype(bf16)
    Wq = np.ascontiguousarray(np.asarray(Wq, dtype=np.float32)).astype(bf16)
    Wkv = np.ascontiguousarray(np.asarray(Wkv, dtype=np.float32)).astype(bf16)
    Wc = np.ascontiguousarray(np.asarray(Wc, dtype=np.float32)).astype(bf16)
    in_maps = []
    for i in range(N_CORES):
        in_maps.append({
            "qT": np.ascontiguousarray(qT_full[:, LS * i:LS * (i + 1)]),
            "kvT": kvT,
            "Wq": Wq,
            "Wkv": Wkv,
            "Wc": Wc,
        })
    return in_maps


def kernel(q, kv, Wq, Wkv, Wc, w):
    assert int(w) == W
    q = np.asarray(q, dtype=np.float32)
    B = q.shape[0]
    assert B == 1 and q.shape[1] == L and q.shape[2] == DM

    in_maps = prep_in_maps(q, kv, Wq, Wkv, Wc)
    nc = _get_nc()
    res = run_bass_kernel_spmd(nc, in_maps, list(range(N_CORES)))
    out = np.concatenate([res.results[i]["out"] for i in range(N_CORES)], axis=0)
    return out.reshape(1, L, DM).astype(np.float32)


# revision 11
# speedup vs baseline: 1.3080x; 1.3080x over previous
"""Trainium2 Bass kernel for LocalXLAttention (chunk-summed variant).

Math: the reference einsum sums over the chunk index z, so every query
attends to the same three [w, dh] K/V matrices built from chunk sums:
  K_prev = S_k - k_chunk[C-1], K_cur = S_k, K_next = S_k - k_chunk[0]
(and identically for V), where S_k = sum_c k_chunk[c].  The computation
collapses to, per sequence position l and head h:
  attn[l,h,:]  = qp[l,h,:] @ KbigT          (KbigT: [dh, 3w])
  probs        = softmax(attn, axis=-1)
  ctx[l,h,:]   = probs[l,h,:] @ Vbig        (Vbig:  [3w, dh])
  out          = ctx.reshape(L, dm) @ Wc

Sharding: L=4096 is split 512 rows per core across 8 NeuronCores
(data-parallel over the sequence; no collectives).  Each core redundantly
computes the tiny chunk-summed K/V from the full kv input.

All matmul operands are bf16 (fp32 PSUM accumulation): fast-weight-load
LDWEIGHTS, half the HBM traffic, half the DVE cost.  The attention
pipeline runs fully transposed ([j, l] / [he, l] layouts); the softmax
denominator comes for free from an all-ones column of Vbig, and the
normalization (reciprocal + rank-1 PE broadcast + multiply) is hidden
under the ACT-bound attention loop.

Schedule: kv stream is consumed per-dm-chunk as it arrives (PE
projections + DVE chunk-sum tree race the DMA); QP_T head-chunks are
interleaved into the attention pair loop, which is exp(ACT)-bound; Wc
weights load during attention; outputs stream per 128-row chunk.
"""

import sys
for _p in ('/opt/pypackages', '/opt/trn_rl_repo'):
    if _p not in sys.path:
        sys.path.insert(0, _p)

import numpy as np
import ml_dtypes

import concourse.bass as bass
import concourse.bacc as bacc
import concourse.tile as tile
from concourse import mybir
from concourse.bass_utils import run_bass_kernel_spmd
from concourse.masks import make_identity

F32 = mybir.dt.float32
BF16 = mybir.dt.bfloat16
AF = mybir.ActivationFunctionType

N_CORES = 8
L = 4096          # full sequence
LS = L // N_CORES # 512 rows per core
DM = 1024
NH = 16
DH = 64
W = 512           # chunk width
C = L // W        # 8 chunks
J3 = 3 * W        # 1536 softmax width
NJ = J3 // 128    # 12 j-chunks
DMT = DM // 128   # 8 dm-chunks
NPAIR = NH // 2   # 8 head pairs


def build_nc():
    nc = bacc.Bacc(None, target_bir_lowering=False)

    qT = nc.dram_tensor("qT", [DM, LS], BF16, kind="ExternalInput")
    kvT = nc.dram_tensor("kvT", [DM, L], BF16, kind="ExternalInput")
    Wq = nc.dram_tensor("Wq", [DM, DM], BF16, kind="ExternalInput")
    Wkv = nc.dram_tensor("Wkv", [DM, 2 * DH], BF16, kind="ExternalInput")
    Wc = nc.dram_tensor("Wc", [DM, DM], BF16, kind="ExternalInput")
    out = nc.dram_tensor("out", [LS, DM], F32, kind="ExternalOutput")

    with tile.TileContext(nc) as tc:
        with tc.tile_pool(name="weights", bufs=8) as wpool, \
             tc.tile_pool(name="small", bufs=1) as spool, \
             tc.tile_pool(name="qp", bufs=8) as qpool, \
             tc.tile_pool(name="stream", bufs=3) as stpool, \
             tc.tile_pool(name="kvsum", bufs=3) as kvspool, \
             tc.tile_pool(name="probs", bufs=2) as ppool, \
             tc.tile_pool(name="misc", bufs=2) as mpool, \
             tc.tile_pool(name="dram", bufs=1, space="DRAM") as dpool, \
             tc.tile_pool(name="psacc", bufs=4, space="PSUM") as psacc, \
             tc.tile_pool(name="psmm", bufs=2, space="PSUM") as psmm:

            ENGS = (nc.sync, nc.scalar, nc.gpsimd)

            # ---------- DMA priority: Wkv+qT, then kv || Wq, Wc last -----
            wkv_sb = []
            for d in range(DMT):
                t = wpool.tile([128, 2 * DH], BF16, tag="wkv", name=f"wkv{d}")
                ENGS[d % 3].dma_start(out=t, in_=Wkv[128 * d:128 * (d + 1), :])
                wkv_sb.append(t)
            qt_sb = []
            for d in range(DMT):
                t = qpool.tile([128, LS], BF16, tag="qt", name=f"qt{d}")
                ENGS[(d + 1) % 3].dma_start(out=t, in_=qT[128 * d:128 * (d + 1), :])
                qt_sb.append(t)

            # zero tile for PE warm-up (DVE memset: not gated on gpsimd)
            zt = spool.tile([128, 128], BF16, tag="zt")
            nc.vector.memset(zt, 0.0)
            ident = spool.tile([128, 128], BF16, tag="ident")
            make_identity(nc, ident)

            # kv stream halves + Wq interleaved across the three queues
            st_sb = []
            for d in range(DMT):
                st = stpool.tile([128, L], BF16, tag="kvstream", name=f"st{d}")
                ENGS[(2 * d) % 3].dma_start(
                    out=st[:, 0:L // 2],
                    in_=kvT[128 * d:128 * (d + 1), 0:L // 2])
                ENGS[(2 * d + 1) % 3].dma_start(
                    out=st[:, L // 2:L],
                    in_=kvT[128 * d:128 * (d + 1), L // 2:L])
                st_sb.append(st)
            wq_sb = []
            for d in range(DMT):
                t = wpool.tile([128, DM], BF16, tag="wq", name=f"wq{d}")
                ENGS[d % 3].dma_start(out=t, in_=Wq[128 * d:128 * (d + 1), :])
                wq_sb.append(t)
            wc_sb = []
            for d in range(DMT):
                t = wpool.tile([128, DM], BF16, tag="wc", name=f"wc{d}")
                ENGS[d % 3].dma_start(out=t, in_=Wc[128 * d:128 * (d + 1), :])
                wc_sb.append(t)

            # ---------- PE warm-up during the DMA wait (HAM clock gate) --
            warm_ps = psacc.tile([128, W], F32, tag="acc", name="warm")
            for i in range(110):
                nc.tensor.matmul(warm_ps[:, 0:128], zt, zt,
                                 start=True, stop=True)
            # preload the exp activation table during startup
            exp_warm = spool.tile([1, 8], F32, tag="expwarm")
            nc.scalar.activation(exp_warm, zt[0:1, 0:8], AF.Exp, scale=1.0)

            # ---------- kv stream: chunk-sum tree + projections ----------
            # PSUM accumulators pack K rows 0:64, V rows 64:128.
            ps0 = psacc.tile([128, W], F32, tag="acc", name="ps0")
            ps7 = psacc.tile([128, W], F32, tag="acc", name="ps7")
            pss = psacc.tile([128, W], F32, tag="acc", name="pss")
            for d in range(DMT):
                st = st_sb[d]
                nc.tensor.matmul(ps0[0:DH, :], wkv_sb[d][:, 0:DH],
                                 st[:, 0:W], start=(d == 0), stop=(d == DMT - 1))
                nc.tensor.matmul(ps0[DH:128, :], wkv_sb[d][:, DH:2 * DH],
                                 st[:, 0:W], start=(d == 0), stop=(d == DMT - 1))
                nc.tensor.matmul(ps7[0:DH, :], wkv_sb[d][:, 0:DH],
                                 st[:, L - W:L], start=(d == 0), stop=(d == DMT - 1))
                nc.tensor.matmul(ps7[DH:128, :], wkv_sb[d][:, DH:2 * DH],
                                 st[:, L - W:L], start=(d == 0), stop=(d == DMT - 1))
                nc.vector.tensor_add(st[:, 0:2048], st[:, 0:2048], st[:, 2048:4096])
                nc.vector.tensor_add(st[:, 0:1024], st[:, 0:1024], st[:, 1024:2048])
                ks = kvspool.tile([128, W], BF16, tag="kvsum")
                nc.vector.tensor_add(ks, st[:, 0:512], st[:, 512:1024])
                nc.tensor.matmul(pss[0:DH, :], wkv_sb[d][:, 0:DH],
                                 ks, start=(d == 0), stop=(d == DMT - 1))
                nc.tensor.matmul(pss[DH:128, :], wkv_sb[d][:, DH:2 * DH],
                                 ks, start=(d == 0), stop=(d == DMT - 1))

            # ---------- evacuate K/V variants to SBUF (bf16) -------------
            kv0_sb = spool.tile([128, W], BF16, tag="kv0")  # K rows 0:64, V 64:128
            kv7_sb = spool.tile([128, W], BF16, tag="kv7")
            kvs_sb = spool.tile([128, W], BF16, tag="kvs")
            nc.vector.tensor_copy(kv0_sb, ps0)
            nc.vector.tensor_copy(kv7_sb, ps7)
            nc.vector.tensor_copy(kvs_sb, pss)

            # ---------- KbigT [128, 1536] = [prev | cur | next] ----------
            kbig = spool.tile([128, J3], BF16, tag="kbig")
            nc.vector.tensor_sub(kbig[0:DH, 0:W], kvs_sb[0:DH, :], kv7_sb[0:DH, :])
            nc.vector.tensor_copy(kbig[0:DH, W:2 * W], kvs_sb[0:DH, :])
            nc.vector.tensor_sub(kbig[0:DH, 2 * W:3 * W], kvs_sb[0:DH, :],
                                 kv0_sb[0:DH, :])
            nc.vector.tensor_copy(kbig[DH:2 * DH, :], kbig[0:DH, :])

            # ---------- Vbig [128, 12, 65(+pad)] -------------------------
            vbig = spool.tile([128, NJ, 68], BF16, tag="vbig")
            ones_sb = spool.tile([128, 1], BF16, tag="ones")
            nc.vector.memset(ones_sb, 1.0)
            for j in range(NJ):
                nc.vector.tensor_copy(vbig[:, j, DH:DH + 1], ones_sb)
            for yt in range(4):
                tps = psacc.tile([128, DH], BF16, tag="acc")
                tp0 = psacc.tile([128, DH], BF16, tag="acc")
                tp7 = psacc.tile([128, DH], BF16, tag="acc")
                sl = slice(128 * yt, 128 * (yt + 1))
                idq = ident[DH:128, DH:128]
                nc.tensor.transpose(tps, kvs_sb[DH:128, sl], idq)
                nc.tensor.transpose(tp0, kv0_sb[DH:128, sl], idq)
                nc.tensor.transpose(tp7, kv7_sb[DH:128, sl], idq)
                nc.vector.tensor_copy(vbig[:, 4 + yt, 0:DH], tps)
                nc.vector.tensor_sub(vbig[:, 0 + yt, 0:DH], vbig[:, 4 + yt, 0:DH], tp7)
                nc.vector.tensor_sub(vbig[:, 8 + yt, 0:DH], vbig[:, 4 + yt, 0:DH], tp0)



            # ---------- QP_T chunks (emitted on demand in the pair loop) -
            qpt_sb = [None] * NPAIR

            def emit_qp(t):
                ps = psmm.tile([128, 1024], F32, tag="mm", name=f"qps{t}")
                for d in range(DMT):
                    nc.tensor.matmul(ps[:, 0:512],
                                     wq_sb[d][:, 128 * t:128 * (t + 1)],
                                     qt_sb[d], start=(d == 0), stop=(d == DMT - 1))
                sb = qpool.tile([128, LS], BF16, tag="qpt", name=f"qpt{t}")
                nc.vector.tensor_copy(sb, ps[:, 0:512])
                qpt_sb[t] = sb

            # ---------- attention: QK -> exp(PSUM direct) -> PV ----------
            ctxu_sb = []  # per pair [128, 512]: rows 0:64 head 2t, 64:128 head 2t+1
            for t in range(NPAIR):
                ctxu_sb.append(qpool.tile([128, W], BF16, tag="ctxu",
                                          name=f"ctxu{t}"))

            rsc = dpool.tile([NPAIR, 2 * W], BF16, name="rsc")

            def normalize(t):
                # broadcast 1/denom (staged in DRAM) across partitions, then
                # scale ctxu in place.  Called one pair late so nothing here
                # is on the critical path.
                bc = mpool.tile([128, W], BF16, tag="bcast", name=f"bc{t}")
                src = bass.AP(tensor=rsc.tensor,
                              offset=rsc.offset + t * 2 * W,
                              ap=[[W, 2], [0, DH], [1, W]])
                nc.gpsimd.dma_start(out=bc, in_=src)
                nc.vector.tensor_mul(ctxu_sb[t], ctxu_sb[t], bc)

            emit_qp(0)
            emit_qp(1)
            for t in range(NPAIR):  # head pairs (2t, 2t+1)
                qpt = qpt_sb[t]
                ctxA = psacc.tile([128, W], F32, tag="acc", name=f"ctxA{t}")
                ctxB = psacc.tile([128, W], F32, tag="acc", name=f"ctxB{t}")
                for j in range(NJ):
                    qk = psmm.tile([128, 1024], F32, tag="mm", name=f"qk{t}_{j}")
                    nc.tensor.matmul(qk[:, 0:W],
                                     kbig[0:DH, 128 * j:128 * (j + 1)],
                                     qpt[0:DH, :], start=True, stop=True)
                    nc.tensor.matmul(qk[:, W:2 * W],
                                     kbig[DH:2 * DH, 128 * j:128 * (j + 1)],
                                     qpt[DH:128, :], start=True, stop=True)
                    pr = ppool.tile([128, 1024], BF16, tag="probs",
                                    name=f"pr{t}_{j}")
                    nc.scalar.activation(pr, qk, AF.Exp, scale=0.125)
                    nc.tensor.matmul(ctxA[0:DH + 1, :], vbig[:, j, 0:DH + 1],
                                     pr[:, 0:W],
                                     start=(j == 0), stop=(j == NJ - 1))
                    nc.tensor.matmul(ctxB[0:DH + 1, :], vbig[:, j, 0:DH + 1],
                                     pr[:, W:2 * W],
                                     start=(j == 0), stop=(j == NJ - 1))
                if t > 0:
                    normalize(t - 1)
                # context (cast bf16) + reciprocal of the denominator rows
                nc.vector.tensor_copy(ctxu_sb[t][0:DH, :], ctxA[0:DH, :])
                nc.vector.tensor_copy(ctxu_sb[t][DH:128, :], ctxB[0:DH, :])
                dtmp = mpool.tile([1, 2 * W], F32, tag="dtmp", name=f"dtmp{t}",
                                  bufs=2)
                nc.vector.tensor_copy(dtmp[:, 0:W], ctxA[DH:DH + 1, :])
                nc.vector.tensor_copy(dtmp[:, W:2 * W], ctxB[DH:DH + 1, :])
                rcf = mpool.tile([1, 2 * W], F32, tag="rcf", name=f"rcf{t}",
                                 bufs=2)
                nc.vector.reciprocal(rcf, dtmp)
                rc16 = mpool.tile([1, 2 * W], BF16, tag="rc16", name=f"rc16{t}",
                                  bufs=2)
                nc.vector.tensor_copy(rc16, rcf)
                nc.sync.dma_start(out=rsc[t:t + 1, :], in_=rc16)
                if t + 2 < NPAIR:
                    emit_qp(t + 2)
            normalize(NPAIR - 1)

            # ---------- out = ctx @ Wc ----------
            for lt in range(LS // 128):
                ps = psmm.tile([128, 1024], F32, tag="mm")
                for half in range(2):
                    for he in range(DMT):
                        nc.tensor.matmul(
                            ps[:, 512 * half:512 * (half + 1)],
                            ctxu_sb[he][:, 128 * lt:128 * (lt + 1)],
                            wc_sb[he][:, 512 * half:512 * (half + 1)],
                            start=(he == 0), stop=(he == DMT - 1))
                ob = mpool.tile([128, DM], F32, tag="outsb", bufs=2)
                nc.vector.tensor_copy(ob, ps)
                nc.sync.dma_start(out=out[128 * lt:128 * (lt + 1), :], in_=ob)

    nc.compile()
    return nc


_NC = None


def _get_nc():
    global _NC
    if _NC is None:
        _NC = build_nc()
    return _NC


def prep_in_maps(q, kv, Wq, Wkv, Wc):
    """Host-side input prep: transpose, cast to bf16, shard queries."""
    bf16 = ml_dtypes.bfloat16
    qT_full = np.ascontiguousarray(np.asarray(q, dtype=np.float32)[0].T
                                   ).astype(bf16)
    kvT = np.ascontiguousarray(np.asarray(kv, dtype=np.float32)[0].T
                               ).astype(bf16)
    Wq = np.ascontiguousarray(np.asarray(Wq, dtype=np.float32)).astype(bf16)
    Wkv = np.ascontiguousarray(np.asarray(Wkv, dtype=np.float32)).astype(bf16)
    Wc = np.ascontiguousarray(np.asarray(Wc, dtype=np.float32)).astype(bf16)
    in_maps = []
    for i in range(N_CORES):
        in_maps.append({
            "qT": np.ascontiguousarray(qT_full[:, LS * i:LS * (i + 1)]),
            "kvT": kvT,
            "Wq": Wq,
            "Wkv": Wkv,
            "Wc": Wc,
        })
    return in_maps


def kernel(q, kv, Wq, Wkv, Wc, w):
    assert int(w) == W
    q = np.asarray(q, dtype=np.float32)
    B = q.shape[0]
    assert B == 1 and q.shape[1] == L and q.shape[2] == DM

    in_maps = prep_in_maps(q, kv, Wq, Wkv, Wc)
    nc = _get_nc()
    res = run_bass_kernel_spmd(nc, in_maps, list(range(N_CORES)))
    out = np.concatenate([res.results[i]["out"] for i in range(N_CORES)], axis=0)
    return out.reshape(1, L, DM).astype(np.float32)
